# revision 23
# baseline (speedup 1.0000x reference)
"""Trainium2 Bass kernel for nn_DeXPaReClassifier (8-core SPMD).

Reference math:
  img_n = l2norm(img_f)*64 ; tex_n = l2norm(tex_f)*64
  attn   = softmax(fake_cls @ fc(tex_n).T) ; tex_a = attn @ tex_n
  cp     = einsum('bxc,pdc->bpxd', img_n, tex_a).reshape(B, 1024)
  h      = elu(bn1(cp) @ w1.T + b1)
  out    = bn2(h) @ w2.T + b2

Split of work:
  Host (exact f32, tiny FLOPs): input normalization, the prompt-attention
  branch (tex_a is 8x16x512), and bn1's batch statistics — cp is linear in
  img_n so mean/var of cp are computed host-side from the same f32 math the
  reference uses, then folded into w1/b1 (w1eff = w1*s1, b1eff = b1+w1@t1).
  Device (per core, batch 1024): cp = taT.T@img (PE), fc1+ELU, bn2 with
  batch stats via two half-feature AllGathers, fc2. ELU's -1 is dropped
  (bn2 is shift invariant).

Distribution: data-parallel over batch, 1024/core. Collectives: bn2's two
stat AllGathers plus a zero-byte warm-up AllGather at t=0 so the one-time
collective bootstrap barrier (~45-55us) overlaps the DMA + cp + fc1 phase.

Stall avoidance (engine queues are strict FIFO, so a waiting instruction
blocks everything behind it on that engine):
  - collective bounce-buffer writes + triggers: gpsimd ring (nothing else)
  - collective result loads: SP/sync ring (idle after the input DMAs)
  - bn2a is processed at fc1 end, after its AllGather has finished
  - fc2 runs as two passes: pass 1 accumulates fo 0..7 (normalized by
    bn2a) for all 16 output tiles and banks partials+bias in SBUF f32 —
    ~35us of PE work bridging the second AllGather — then pass 2 adds
    fo 8..15 and the banked partial.

On-device layout: feature-on-partition (transposed), bf16 GEMM operands,
f32 PSUM. cp feature order is (x, p, a); w1eff is host-permuted to match.
DMA rings: img (per-co chunks) + w2 (one resident transfer) on sync ring;
taT + small params + w1 stream + out writes on the ACT ring.
"""
import numpy as np
import ml_dtypes
from contextlib import ExitStack

import concourse.bass as bass
import concourse.tile as tile
from concourse import bacc, mybir
from concourse.bass_utils import run_bass_kernel_spmd

F32 = mybir.dt.float32
F16 = mybir.dt.float16
BF16 = mybir.dt.bfloat16
AF = mybir.ActivationFunctionType
OP = mybir.AluOpType

N_CORES = 8
B, X, C = 8192, 8, 512
P, Y, A = 8, 1000, 16
IN_DIM = 1024        # P*X*A
F = 2048
CLS = 1000
CLSP = 1024          # CLS padded
BL = B // N_CORES    # 1024 batch per core
SF = 64.0
EPS_BN = 1e-5

_CACHE = {}


def build():
    nc = bacc.Bacc(None, target_bir_lowering=False, debug=False, num_devices=N_CORES)

    # ---- parameters (per-core values supplied via in_maps)
    imgT = nc.declare_dram_parameter("imgT", [X, C, BL], BF16, isOutput=False)
    taT = nc.declare_dram_parameter("taT", [C, 128], BF16, isOutput=False)
    w1H = nc.declare_dram_parameter("w1H", [2, 128, 8, F // 2], BF16, isOutput=False)
    b1t = nc.declare_dram_parameter("b1t", [128, 16], F32, isOutput=False)
    w2T = nc.declare_dram_parameter("w2T", [F, CLSP], BF16, isOutput=False)
    g2t = nc.declare_dram_parameter("g2t", [128, 16], F32, isOutput=False)
    b2bt = nc.declare_dram_parameter("b2bt", [128, 16], F32, isOutput=False)
    b2t = nc.declare_dram_parameter("b2t", [128, 8], F32, isOutput=False)
    outT = nc.declare_dram_parameter("outT", [CLSP, BL], F32, isOutput=True)

    # ---- internal DRAM for collectives
    ar0_in = nc.dram_tensor("ar0_in", [1, 8], F32)
    ar0_out = nc.dram_tensor("ar0_out", [N_CORES, 8], F32, addr_space="Shared")
    ar2a_in = nc.dram_tensor("ar2a_in", [128, 16], F16)
    ar2a_out = nc.dram_tensor("ar2a_out", [128 * N_CORES, 16], F16, addr_space="Shared")
    ar2b_in = nc.dram_tensor("ar2b_in", [128, 16], F16)
    ar2b_out = nc.dram_tensor("ar2b_out", [128 * N_CORES, 16], F16, addr_space="Shared")
    RG = [list(range(N_CORES))]

    with ExitStack() as ctx:
        tc = ctx.enter_context(tile.TileContext(nc))
        # pools
        pimg = ctx.enter_context(tc.tile_pool(name="pimg", bufs=7))
        pta = ctx.enter_context(tc.tile_pool(name="pta", bufs=1))
        pw = ctx.enter_context(tc.tile_pool(name="pw", bufs=2))
        pw2 = ctx.enter_context(tc.tile_pool(name="pw2", bufs=1))
        pcp = ctx.enter_context(tc.tile_pool(name="pcp", bufs=1))
        ph = ctx.enter_context(tc.tile_pool(name="ph", bufs=1))
        psb = ctx.enter_context(tc.tile_pool(name="psb", bufs=3))
        psm = ctx.enter_context(tc.tile_pool(name="psm", bufs=1))
        pout = ctx.enter_context(tc.tile_pool(name="pout", bufs=3))
        psum = ctx.enter_context(tc.tile_pool(name="psA", bufs=4, space="PSUM"))

        # ---------------- warm-up collective: absorb the one-time barrier --
        nc.gpsimd.collective_compute("AllGather", OP.bypass, replica_groups=RG,
                                     ins=[ar0_in.ap().opt()],
                                     outs=[ar0_out.ap().opt()])

        # ---------------- taT first on ACT ring, then small params ---------
        ta = pta.tile([128, 4, 128], BF16, tag="ta")
        nc.scalar.dma_start(
            out=ta, in_=taT.ap().rearrange("(co cp) a -> cp co a", cp=128))
        b1c = psm.tile([128, 16], F32, tag="b1c")
        nc.scalar.dma_start(out=b1c, in_=b1t[:, :])
        g2 = psm.tile([128, 16], F32, tag="g2")
        nc.scalar.dma_start(out=g2, in_=g2t[:, :])
        b2b = psm.tile([128, 16], F32, tag="b2b")
        nc.scalar.dma_start(out=b2b, in_=b2bt[:, :])
        b2c = psm.tile([128, 8], F32, tag="b2c")
        nc.scalar.dma_start(out=b2c, in_=b2t[:, :])
        epsc = psm.tile([128, 1], F32, tag="epsc")
        nc.vector.memset(epsc, EPS_BN)

        # ---------------- img DMAs (sync ring), per-co chunks --------------
        img_tiles = []
        for x in range(X):
            ti = pimg.tile([128, 4, BL], BF16, tag="ti", name=f"ti{x}")
            r = imgT[x].rearrange("(co cp) b -> cp co b", cp=128)
            for co in range(4):
                nc.sync.dma_start(out=ti[:, co, :], in_=r[:, co, :])
            img_tiles.append(ti)

        # ---------------- w1 resident via two host-packed half loads on the
        # sync ring AFTER img (sequential on the same ring: never steals the
        # early HBM window; 128 contiguous 16KB descriptors each; first half
        # lands right as cp drains so fc1 is never load-paced) --------------
        w1_halves = []
        for hf in range(2):
            wh = pw.tile([128, 8, F // 2], BF16, tag="w1f", name=f"w1f{hf}")
            nc.sync.dma_start(out=wh, in_=w1H[hf])
            w1_halves.append(wh)

        # ---------------- w2 resident (one transfer, sync ring) ------------
        w2t = pw2.tile([128, 16, CLSP], BF16, tag="w2t")
        nc.sync.dma_start(
            out=w2t, in_=w2T.ap().rearrange("(fp2 fpp) y -> fpp fp2 y", fpp=128))

        # ---------------- cp = taT.T @ img ---------------------------------
        cp = pcp.tile([128, X, BL], BF16, tag="cp")
        with nc.named_scope("cp"):
            for x in range(X):
                ti = img_tiles[x]
                pcs = [psum.tile([128, 512], F32, tag="cp", bufs=4,
                                 name=f"pc{x}_{bh}") for bh in range(2)]
                for co in range(4):
                    for bh in range(2):
                        nc.tensor.matmul(pcs[bh], ta[:, co, :],
                                         ti[:, co, bh * 512:(bh + 1) * 512],
                                         start=(co == 0), stop=(co == 3))
                for bh in range(2):
                    nc.scalar.activation(cp[:, x, bh * 512:(bh + 1) * 512],
                                         pcs[bh], AF.Copy)

        # ---------------- fc1 + ELU(+1) + bn2 stats ------------------------
        h = ph.tile([128, 16, BL], BF16, tag="h")
        st2 = psm.tile([128, 16, 2, 6], F32, tag="st2")
        mv2 = psm.tile([128, 16, 2], F32, tag="mv2")
        es2a = psm.tile([128, 2, 8], F16, tag="es2a")
        es2b = psm.tile([128, 2, 8], F16, tag="es2b")
        s2 = psm.tile([128, 16], F32, tag="s2")
        t2 = psm.tile([128, 16], F32, tag="t2")

        def fc1_evict(fo, bh, phm):
            # elu+1: h = min(exp(y),1) + relu(y),  y = psum + b1
            te = psb.tile([128, 512], BF16, tag="te")
            nc.scalar.activation(te, phm, AF.Exp, bias=b1c[:, fo:fo + 1])
            tr = psb.tile([128, 512], BF16, tag="tr")
            nc.scalar.activation(tr, phm, AF.Relu, bias=b1c[:, fo:fo + 1])
            nc.vector.scalar_tensor_tensor(h[:, fo, bh * 512:(bh + 1) * 512],
                                           te, 1.0, tr, OP.min, OP.add)
            nc.vector.bn_stats(st2[:, fo, bh, :],
                               h[:, fo, bh * 512:(bh + 1) * 512])

        def fc1_stats(fo):
            nc.vector.bn_aggr(mv2[:, fo, :], st2[:, fo, :, :])
            if fo == 7 or fo == 15:
                # per-core [mean, var] in fp16 — halves the AllGather payload
                # (its latency is roughly linear in bytes); costs ~2e-4 rel
                es = es2a if fo == 7 else es2b
                lo = fo - 7
                nc.vector.tensor_copy(es[:, 0, :], mv2[:, lo:lo + 8, 0])
                nc.vector.tensor_copy(es[:, 1, :], mv2[:, lo:lo + 8, 1])
                arin = ar2a_in if fo == 7 else ar2b_in
                arout = ar2a_out if fo == 7 else ar2b_out
                # bounce write + trigger both on the otherwise-empty gpsimd
                # ring so they are never queued behind blocked instructions
                nc.gpsimd.dma_start(out=arin[:, :],
                                    in_=es.rearrange("p t f -> p (t f)"))
                nc.gpsimd.collective_compute(
                    "AllGather", OP.bypass, replica_groups=RG,
                    ins=[arin.ap().opt()], outs=[arout.ap().opt()])

        def bn_half(arout, lo, esname):
            """Reduce gathered per-core fp16 [mean, var] -> bn2 scale/shift.

            var_global = E[var_i] + E[mean_i^2] - E[mean_i]^2. The gathered-
            result load goes on the sync ring (idle after the input DMAs) so
            its collective-wait never blocks other work."""
            ag = psm.tile([128, N_CORES, 16], F16, tag=esname + "_ag",
                          name=esname + "_ag")
            nc.sync.dma_start(
                out=ag, in_=arout.ap().rearrange("(r p) s -> p r s", p=128))
            agm = ag.rearrange("p r (t f) -> p t f r", t=2)
            arm = psm.tile([128, 8], F32, tag=esname + "_m", name=esname + "_m")
            nc.vector.tensor_reduce(arm, agm[:, 0, :, :],
                                    mybir.AxisListType.X, OP.add)
            arv = psm.tile([128, 8], F32, tag=esname + "_w", name=esname + "_w")
            nc.vector.tensor_reduce(arv, agm[:, 1, :, :],
                                    mybir.AxisListType.X, OP.add)
            sqm = psm.tile([128, 8, N_CORES], F32, tag=esname + "_q",
                           name=esname + "_q")
            nc.vector.tensor_tensor(sqm, agm[:, 0, :, :], agm[:, 0, :, :],
                                    OP.mult)
            arm2 = psm.tile([128, 8], F32, tag=esname + "_2", name=esname + "_2")
            nc.vector.tensor_reduce(arm2, sqm, mybir.AxisListType.X, OP.add)
            e = psm.tile([128, 8], F32, tag=esname + "_e", name=esname + "_e")
            nc.vector.tensor_scalar_mul(e, arm, 1.0 / N_CORES)
            var = psm.tile([128, 8], F32, tag=esname + "_v", name=esname + "_v")
            nc.vector.scalar_tensor_tensor(var, e, 1.0, e, OP.mult, OP.mult)
            nc.vector.scalar_tensor_tensor(var, arm2, 1.0 / N_CORES, var,
                                           OP.mult, OP.subtract)
            nc.vector.scalar_tensor_tensor(var, arv, 1.0 / N_CORES, var,
                                           OP.mult, OP.add)
            sl = slice(lo, lo + 8)
            nc.scalar.activation(s2[:, sl], var, AF.Abs_reciprocal_sqrt,
                                 bias=epsc)
            nc.vector.tensor_tensor(s2[:, sl], s2[:, sl], g2[:, sl], OP.mult)
            nc.vector.scalar_tensor_tensor(t2[:, sl], e, -1.0, s2[:, sl],
                                           OP.mult, OP.mult)
            nc.vector.tensor_tensor(t2[:, sl], t2[:, sl], b2b[:, sl], OP.add)

        def bn2_apply(fo):
            nc.vector.tensor_scalar(h[:, fo, :], h[:, fo, :],
                                    s2[:, fo:fo + 1], t2[:, fo:fo + 1],
                                    OP.mult, OP.add)

        with nc.named_scope("fc1"):
            for fo in range(16):
                wt = w1_halves[fo // 8]
                fl = (fo % 8) * 128
                phs = [psum.tile([128, 512], F32, tag="mm",
                                 name=f"phm{fo}_{bh}") for bh in range(2)]
                for go in range(8):
                    for bh in range(2):
                        nc.tensor.matmul(phs[bh], wt[:, go, fl:fl + 128],
                                         cp[:, go, bh * 512:(bh + 1) * 512],
                                         start=(go == 0), stop=(go == 7))
                for bh in range(2):
                    fc1_evict(fo, bh, phs[bh])
                fc1_stats(fo)

        # bn2 first half: AG-a finished during late fc1; process + apply now
        with nc.named_scope("bn2a"):
            bn_half(ar2a_out, 0, "e2a")
            for fa in range(8):
                bn2_apply(fa)

        # ---------------- fc2: two passes bridging AG-b --------------------
        P1 = psm.tile([128, 16, 512], BF16, tag="P1")
        with nc.named_scope("fc2a"):
            for yo in range(8):
                for bh in range(2):
                    i = yo * 2 + bh
                    po = psum.tile([128, 512], F32, tag=("mm" if i % 2 == 0 else "cp"),
                                   bufs=4, name=f"p1_{yo}_{bh}")
                    for fo in range(8):
                        nc.tensor.matmul(po, w2t[:, fo, yo * 128:(yo + 1) * 128],
                                         h[:, fo, bh * 512:(bh + 1) * 512],
                                         start=(fo == 0), stop=(fo == 7))
                    nc.scalar.activation(P1[:, i, :], po, AF.Identity,
                                         bias=b2c[:, yo:yo + 1])

        with nc.named_scope("bn2b"):
            bn_half(ar2b_out, 8, "e2b")
            for fo in range(8, 16):
                bn2_apply(fo)

        with nc.named_scope("fc2b"):
            for yo in range(8):
                for bh in range(2):
                    i = yo * 2 + bh
                    po = psum.tile([128, 512], F32, tag=("mm" if i % 2 == 0 else "cp"),
                                   bufs=4, name=f"p2_{yo}_{bh}")
                    for fo in range(8, 16):
                        nc.tensor.matmul(po, w2t[:, fo, yo * 128:(yo + 1) * 128],
                                         h[:, fo, bh * 512:(bh + 1) * 512],
                                         start=(fo == 8), stop=(fo == 15))
                    to = pout.tile([128, 512], F32, tag="to")
                    nc.vector.tensor_tensor(to, po, P1[:, i, :], OP.add)
                    nc.scalar.dma_start(
                        out=outT.ap()[yo * 128:(yo + 1) * 128,
                                      bh * 512:(bh + 1) * 512],
                        in_=to)
    nc.compile()
    return nc


def _get_nc():
    if "nc" not in _CACHE:
        _CACHE["nc"] = build()
    return _CACHE["nc"]


def _prep_host(inputs):
    img_f = np.asarray(inputs["img_f"], np.float32)
    tex_f = np.asarray(inputs["tex_f"], np.float32)
    fake_cls = np.asarray(inputs["fake_cls"], np.float32)
    fc_w = np.asarray(inputs["fc_w"], np.float32)
    fc_b = np.asarray(inputs["fc_b"], np.float32)
    bn1_g = np.asarray(inputs["bn1_g"], np.float32)
    bn1_b = np.asarray(inputs["bn1_b"], np.float32)
    w1 = np.asarray(inputs["w1"], np.float32)
    b1 = np.asarray(inputs["b1"], np.float32)
    bn2_g = np.asarray(inputs["bn2_g"], np.float32)
    bn2_b = np.asarray(inputs["bn2_b"], np.float32)
    w2 = np.asarray(inputs["w2"], np.float32)
    b2 = np.asarray(inputs["b2"], np.float32)

    # ---- host: normalization + prompt attention (same f32 math as ref)
    img_n = img_f / (np.linalg.norm(img_f, axis=-1, keepdims=True) + 1e-6) * SF
    tex_n = tex_f / (np.linalg.norm(tex_f, axis=-1, keepdims=True) + 1e-6) * SF
    tex_fc = tex_n @ fc_w.T + fc_b                       # (P,Y,C)
    lg = np.matmul(fake_cls, tex_fc.transpose(0, 2, 1))  # (P,16,Y)
    lg -= lg.max(-1, keepdims=True)
    el = np.exp(lg)
    attn = el / el.sum(-1, keepdims=True)
    tex_a = np.matmul(attn, tex_n)                       # (P,16,C)
    taR = tex_a.reshape(P * A, C)                        # pa = p*16+a

    # ---- host: bn1 batch stats of cp (cp is linear in img_n), fold into w1
    imgx = np.ascontiguousarray(img_n.transpose(1, 0, 2))    # (X,B,C)
    cpx = np.matmul(imgx, taR.T)                             # (X,B,128)
    m_x = cpx.mean(1)                                        # (X,128)
    v_x = cpx.var(1)                                         # (X,128)
    # natural feature order of reference cp is (p,x,a)
    m_nat = m_x.reshape(X, P, A).transpose(1, 0, 2).reshape(IN_DIM)
    v_nat = v_x.reshape(X, P, A).transpose(1, 0, 2).reshape(IN_DIM)
    s1 = bn1_g / np.sqrt(v_nat + EPS_BN)
    t1 = bn1_b - m_nat * s1
    w1eff = w1 * s1[None, :]
    b1eff = b1 + w1 @ t1

    # ---- device layouts
    w1perm = w1eff.reshape(F, P, X, A).transpose(0, 2, 1, 3).reshape(F, IN_DIM)
    w1Tb = np.ascontiguousarray(w1perm.T).astype(ml_dtypes.bfloat16)
    # two per-partition-contiguous halves: [half, gp, go, f_within_half]
    w1Hh = np.ascontiguousarray(
        w1Tb.reshape(8, 128, 2, F // 2).transpose(2, 1, 0, 3))
    b1t = np.ascontiguousarray(b1eff.reshape(16, 128).T)
    taT = np.ascontiguousarray(taR.T).astype(ml_dtypes.bfloat16)  # (C,128)
    w2pad = np.zeros((CLSP, F), np.float32)
    w2pad[:CLS] = w2
    w2T = np.ascontiguousarray(w2pad.T).astype(ml_dtypes.bfloat16)
    g2t = np.ascontiguousarray(bn2_g.reshape(16, 128).T)
    b2bt = np.ascontiguousarray(bn2_b.reshape(16, 128).T)
    b2pad = np.zeros((CLSP,), np.float32)
    b2pad[:CLS] = b2
    b2t = np.ascontiguousarray(b2pad.reshape(8, 128).T)

    in_maps = []
    for r in range(N_CORES):
        sh = img_n[r * BL:(r + 1) * BL]                      # (BL, X, C)
        imgTr = np.ascontiguousarray(sh.transpose(1, 2, 0)).astype(ml_dtypes.bfloat16)
        in_maps.append({
            "imgT": imgTr, "taT": taT, "w1H": w1Hh, "b1t": b1t,
            "w2T": w2T, "g2t": g2t, "b2bt": b2bt, "b2t": b2t,
        })
    return in_maps


def kernel(**inputs) -> np.ndarray:
    nc = _get_nc()
    in_maps = _prep_host(inputs)
    res = None
    for attempt in range(3):
        try:
            res = run_bass_kernel_spmd(nc, in_maps, core_ids=list(range(N_CORES)))
            break
        except Exception:
            if attempt == 2:
                raise
            import time
            time.sleep(20)
    out = np.empty((B, CLS), np.float32)
    for r in range(N_CORES):
        out[r * BL:(r + 1) * BL] = res.results[r]["outT"][:CLS].T
    return out
